# revision 24
# baseline (speedup 1.0000x reference)
"""Causal self-attention (B=2, T=2048, C=1024, nh=16) on 8 TRN2 NeuronCores.

Sharding: core c -> batch b = c//4, head group g = c%4 (4 heads each).
Each core computes QKV projections for its heads, causal attention, and a
partial output projection (W_proj rows for its heads). The four partials per
batch are summed on the host, which also adds b_proj.

Layouts (per core, hardcoded):
  xt    [128, 8, 2048]     x[b].T tiles:  xt[p, kt, t] = x[b, t, kt*128+p]
  wqkv  [128, 8, 6, 128]   W_attn q|k|v cols for this core's heads
  bqk   [128, 4] f32       b_attn q|k (per-partition bias)
  bv    [1, 256] f32       b_attn v (broadcast on device)
  wp    [128, 2, 1024]     W_proj rows for this core's heads
  out   [2048, 1024] bf16  partial (x[b] @ ... for this head group)

In-kernel dataflow (all matmuls bf16 with fp32 PSUM accumulation), emitted
tb-block-outer so attention for i-block ib starts right after phase-1 block
tb=ib finishes:
  qT,kT = (W.T @ x.T)      [feat, t] layout  (lhsT=W tile, rhs=xT)
  v     = (x @ Wv)         [t, feat] layout  (lhsT=xT t-tile, rhs=Wv cols)
          -> written straight into vext (65-col per head, ones column)
  S^T   = k @ q.T          [j, i] layout     (lhsT=kT tile, rhs=qT)
  P^T   = exp(S^T/8), masked on diagonal tiles (mult by 0/1 triangle)
  y^T,l = [v|1].T @ P^T    [d, i] layout, row 64 = l = sum_j P
  yT    = y^T * (1/l broadcast)
  out   = yT.T @ Wp        (bias added on host)
"""

import os
import sys

sys.path.insert(0, "/opt/trn_rl_repo")
os.environ.setdefault("MYCRO_LOCAL_CACHE", "1")

import ml_dtypes
import numpy as np

import concourse.bass as bass
import concourse.mybir as mybir
import concourse.tile as tile
from concourse import bacc
from concourse.bass_utils import run_bass_kernel_spmd

B, T, C, NH, HS = 2, 2048, 1024, 16, 64
HPC = 4  # heads per core
N_CORES = 8
KT = C // 128  # 8 contraction tiles over C
TT = T // 128  # 16 tiles over T
IB = T // 512  # 4 i-blocks over T
F32 = mybir.dt.float32

CD = mybir.dt.bfloat16
CD_NP = ml_dtypes.bfloat16

LAST_RESULT = None
_CACHE = {}


def _emit(nc, tc, ctx, aps):
    xt, wqkv, bqk, bv, wp, out = (
        aps["xt"], aps["wqkv"], aps["bqk"], aps["bv"], aps["wp"], aps["out"],
    )
    Exp = mybir.ActivationFunctionType.Exp

    consts = ctx.enter_context(tc.tile_pool(name="consts", bufs=1))

    # --- PE warm-up: full-K dummy matmuls keep HAM busy from t~0 so real
    # matmuls run at 2.4 GHz as soon as their input DMAs land. (K<128
    # row-masked matmuls do NOT count as PE-busy for the HAM clock gate.)
    dummy = consts.tile([128, 512], CD, tag="dummy")
    nc.vector.memset(dummy[:], 1.0)

    # --- persistent SBUF tensors. DMAs are split into ~256-512KB pieces and
    # issued round-robin so all 8 DMA rings transfer in parallel (a single
    # ring moves only ~50-70 GB/s; one 1.5MB DMA alone costs 20-30us).
    # tb=0 pieces and weights go first so phase-1 can start at ~8us.
    wqkv_s = consts.tile([128, KT, 6, 128], CD, tag="wqkv")
    xt_s = consts.tile([128, KT, T], CD, tag="xt")
    bqk_s = consts.tile([128, 4], F32, tag="bqk")
    bv_row = consts.tile([1, 256], F32, tag="bv_row")
    wp_s = consts.tile([128, 2, C], CD, tag="wp")
    for kq in range(4):
        ks = slice(2 * kq, 2 * kq + 2)
        nc.sync.dma_start(xt_s[:, ks, 0:512], xt[:, ks, 0:512])
        nc.sync.dma_start(wqkv_s[:, ks], wqkv[:, ks])
    # small tensors ride right behind the first wave; tb>=1 x chunks after
    nc.sync.dma_start(bqk_s[:], bqk)
    nc.sync.dma_start(bv_row[:], bv)
    nc.sync.dma_start(wp_s[:], wp)
    for tb in range(1, IB):
        tsl = slice(tb * 512, (tb + 1) * 512)
        for hf in range(2):
            ks = slice(4 * hf, 4 * hf + 4)
            nc.sync.dma_start(xt_s[:, ks, tsl], xt[:, ks, tsl])

    # fixed 128x128 causal triangle (keep where j <= c) for diagonal strips
    tri = consts.tile([128, 128], CD, tag="tri")
    nc.vector.memset(tri[:], 1.0)
    nc.gpsimd.affine_select(
        out=tri[:],
        in_=tri[:],
        compare_op=mybir.AluOpType.is_ge,
        fill=0.0,
        base=0,
        channel_multiplier=-1,
        pattern=[[1, 128]],
    )

    qk_t = [consts.tile([128, T], CD, tag=f"q{jt}", name=f"q{jt}")
            for jt in range(2)]
    # kT per head, zero-padded to full 128 partitions: head h occupies rows
    # (h%2)*64..+64, the other 64 rows stay zero. Full-K S-matmuls keep the
    # PE HAM clock-gate warm (K=64 row-masked MMs don't count as PE-busy).
    kz_t = [consts.tile([128, T], CD, tag=f"kz{h}", name=f"kz{h}")
            for h in range(HPC)]
    for h in range(HPC):
        row = ((h + 1) % 2) * 64  # rows NOT written by the k projection
        eng = nc.vector if h < 2 else nc.gpsimd
        eng.memset(kz_t[h][row:row + 64, :], 0.0)
    vext_s = consts.tile([128, TT, HPC * (HS + 1)], CD, tag="vext")
    vext4 = vext_s[:].rearrange("p t (h c) -> p t h c", c=HS + 1)
    nc.gpsimd.memset(vext4[:, :, :, HS], 1.0)  # ones columns
    yt_s = consts.tile([128, 2, T], CD, tag="yt")
    # broadcast LAST on the gpsimd queue: it waits on the bv DMA, and the
    # in-order queue would otherwise block tri/memsets behind it
    bv_bc = consts.tile([128, 256], F32, tag="bv_bc")
    nc.gpsimd.partition_broadcast(bv_bc[:], bv_row[:], channels=128)

    # pools: mm512 is shared by phase-1 QKV groups and the proj matmuls
    mm512 = ctx.enter_context(tc.tile_pool(name="mm512", bufs=2, space="PSUM"))
    attn_sp = ctx.enter_context(tc.tile_pool(name="attn_s", bufs=2, space="PSUM"))
    attn_yp = ctx.enter_context(tc.tile_pool(name="attn_y", bufs=2, space="PSUM"))
    pt_pool = ctx.enter_context(tc.tile_pool(name="pt", bufs=6))
    misc = ctx.enter_context(tc.tile_pool(name="misc", bufs=4))
    stage = ctx.enter_context(tc.tile_pool(name="stage", bufs=3))

    # warm-up matmuls (K=128, N=512): ~3.5us of PE busy before real work
    warm_ps = mm512.tile([128, 512], F32, tag="mm")
    for _ in range(8):
        nc.tensor.matmul(
            out=warm_ps[:], lhsT=dummy[:, 0:128], rhs=dummy[:],
            start=True, stop=True,
        )
    # tiny exp up front pulls the ~2.7us ACT_TABLE_LOAD into the DMA wait
    act_warm = misc.tile([1, 16], CD, tag="actw")
    nc.scalar.activation(out=act_warm[:], in_=dummy[0:1, 0:16], func=Exp)

    def ph1_units(tb):
        """Phase-1 block for token block tb, split into half-group units
        (4 matmuls each) that can braid into attention's exp-paced windows.
        k groups first so the k->bias-add->S chain overlaps the q groups."""
        tsl = slice(tb * 512, (tb + 1) * 512)
        units = []

        def qk_unit(jt, half, cell):
            def emit():
                if half == 0:
                    cell["ps"] = mm512.tile(
                        [128, 512], F32, tag="mm", name=f"qk{tb}_{jt}"
                    )
                ps = cell["ps"]
                for kt in range(4 * half, 4 * half + 4):
                    nc.tensor.matmul(
                        out=ps[:],
                        lhsT=wqkv_s[:, kt, jt, :],
                        rhs=xt_s[:, kt, tsl],
                        start=(kt == 0),
                        stop=(kt == KT - 1),
                    )
                if half == 0:
                    return
                if jt < 2:  # q
                    nc.vector.tensor_scalar_add(
                        qk_t[jt][:, tsl], ps[:], bqk_s[:, jt:jt + 1]
                    )
                else:  # k -> zero-padded per-head kz
                    nc.vector.tensor_scalar_add(
                        kz_t[2 * (jt - 2)][0:64, tsl],
                        ps[0:64, :],
                        bqk_s[0:64, jt:jt + 1],
                    )
                    nc.vector.tensor_scalar_add(
                        kz_t[2 * (jt - 2) + 1][64:128, tsl],
                        ps[64:128, :],
                        bqk_s[64:128, jt:jt + 1],
                    )
            return emit

        def v_unit(tt, half, cell):
            def emit():
                if half == 0:
                    cell["ps"] = mm512.tile(
                        [128, 512], F32, tag="mm", name=f"v{tt}"
                    )
                ps = cell["ps"]
                for kt in range(4 * half, 4 * half + 4):
                    nc.tensor.matmul(
                        out=ps[:, 0:256],
                        lhsT=xt_s[:, kt, tt * 128:(tt + 1) * 128],
                        rhs=wqkv_s[:, kt, 4:6, :].rearrange("p a b -> p (a b)"),
                        start=(kt == 0),
                        stop=(kt == KT - 1),
                    )
                if half == 1:
                    nc.vector.tensor_tensor(
                        out=vext4[:, tt, :, 0:HS],
                        in0=ps[:, 0:256].rearrange("p (h c) -> p h c", c=HS),
                        in1=bv_bc[:].rearrange("p (h c) -> p h c", c=HS),
                        op=mybir.AluOpType.add,
                    )
            return emit

        for jt in (2, 3, 0, 1):  # 2,3 = k; 0,1 = q
            cell = {}
            units += [qk_unit(jt, 0, cell), qk_unit(jt, 1, cell)]
        for tt in range(tb * 4, tb * 4 + 4):
            cell = {}
            units += [v_unit(tt, 0, cell), v_unit(tt, 1, cell)]
        return units

    def emit_proj_block(ib):
        for tloc in range(4):
            ttp = ib * 4 + tloc
            st = stage.tile([128, C], CD, tag="st")
            for eb in range(2):
                psp = mm512.tile([128, 512], F32, tag="mm")
                for dt in range(2):
                    nc.tensor.matmul(
                        out=psp[:],
                        lhsT=yt_s[:, dt, ttp * 128:(ttp + 1) * 128],
                        rhs=wp_s[:, dt, eb * 512:(eb + 1) * 512],
                        start=(dt == 0),
                        stop=(dt == 1),
                    )
                nc.vector.tensor_copy(st[:, eb * 512:(eb + 1) * 512], psp[:])
            nc.sync.dma_start(out[ttp * 128:(ttp + 1) * 128, :], st[:])

    def emit_proj_dt(ib, dt, row_off):
        """Half-projection (one dt = one head pair) for the last i-block.
        dt=0 can run right after head 1 finishes, so only dt=1's eight
        matmuls remain after the final head. The dt=0 partial lands in extra
        output rows [T:T+512]; the host folds it in. Per-eb DMAs drain the
        tail on two rings."""
        for tloc in range(4):
            ttp = ib * 4 + tloc
            st = stage.tile([128, C], CD, tag="st")
            for eb in range(2):
                psp = mm512.tile([128, 512], F32, tag="mm")
                nc.tensor.matmul(
                    out=psp[:],
                    lhsT=yt_s[:, dt, ttp * 128:(ttp + 1) * 128],
                    rhs=wp_s[:, dt, eb * 512:(eb + 1) * 512],
                    start=True,
                    stop=True,
                )
                nc.vector.tensor_copy(st[:, eb * 512:(eb + 1) * 512], psp[:])
                nc.sync.dma_start(
                    out[row_off + ttp * 128:row_off + (ttp + 1) * 128,
                        eb * 512:(eb + 1) * 512],
                    st[:, eb * 512:(eb + 1) * 512],
                )

    def emit_pv_quad(layout, pt, psy, h, last_quad):
        for idx, (j, c, w, off) in enumerate(layout):
            nc.tensor.matmul(
                out=psy[:, off:512],
                lhsT=vext4[:, j, h, :],
                rhs=pt[:, c:c + w],
                start=(j == 0),
                stop=(last_quad and idx == len(layout) - 1),
            )

    def emit_attn_block(ib, fill):
        isl = slice(ib * 512, (ib + 1) * 512)
        njt = 4 * ib + 4
        for h in range(HPC):
            jt_q = h // 2
            row = (h % 2) * 64
            psy = attn_yp.tile([HS + 1, 512], F32, tag="y")
            prev = None
            # one exp per pair of j-tiles (2-bank PSUM read); PV of the
            # previous pair runs on the PE while this pair's exp is on the
            # scalar engine
            for g in range(njt // 2):
                psS = attn_sp.tile([128, 1024], F32, tag="s")
                pt = pt_pool.tile([128, 1024], CD, tag="pt")
                js = [2 * g, 2 * g + 1]
                layout = []
                cs = 0
                for j in js:
                    off = max(0, 128 * j - 512 * ib)
                    w = 512 - off
                    if (cs % 512) + w > 512:  # keep each MM within one bank
                        cs = (cs + 511) // 512 * 512
                    nc.tensor.matmul(
                        out=psS[:, cs:cs + w],
                        lhsT=kz_t[h][:, j * 128:(j + 1) * 128],
                        rhs=qk_t[jt_q][:, ib * 512 + off:(ib + 1) * 512],
                        start=True,
                        stop=True,
                    )
                    layout.append((j, cs, w, off))
                    cs += w
                nc.scalar.activation(
                    out=pt[:, 0:cs], in_=psS[:, 0:cs], func=Exp, scale=0.125,
                )
                for (j, c, w, off) in layout:
                    if 128 * j >= 512 * ib:  # diagonal tile -> mask boundary
                        nc.vector.tensor_mul(
                            pt[:, c:c + 128], pt[:, c:c + 128], tri[:],
                        )
                if prev is not None:
                    emit_pv_quad(prev[1], prev[0], psy, h, False)
                fill()
                prev = (pt, layout)
            emit_pv_quad(prev[1], prev[0], psy, h, True)
            lrow = misc.tile([1, 512], F32, tag="lrow")
            nc.vector.tensor_copy(lrow[:], psy[HS:HS + 1, :])
            linv = misc.tile([1, 512], F32, tag="linv")
            nc.vector.reciprocal_approx_fast(linv[:], lrow[:])
            lbc = misc.tile([64, 512], F32, tag="lbc")
            nc.gpsimd.partition_broadcast(lbc[:], linv[:], channels=64)
            nc.vector.tensor_mul(
                yt_s[row:row + 64, jt_q, isl], psy[0:HS, :], lbc[:]
            )
            if h == 0 and ib > 0:
                # previous i-block's projection: its yt is long-ready, and
                # these dense full-K matmuls fill the exp-paced PE windows
                emit_proj_block(ib - 1)
            if h == 1 and ib == IB - 1:
                emit_proj_dt(IB - 1, 0, 512)

    # interleave: phase-1 block 0 up front, then attention i-block ib with
    # phase-1 block ib+1 braided into its exp-paced PE windows
    fq = []

    def fill():
        if fq:
            fq.pop(0)()

    for i, u in enumerate(ph1_units(0)):
        u()
        if i % 2 == 1 and i < 8:
            wfill = mm512.tile([128, 512], F32, tag="mm", name=f"wf{i}")
            nc.tensor.matmul(
                out=wfill[:], lhsT=dummy[:, 0:128], rhs=dummy[:],
                start=True, stop=True,
            )
    for ib in range(IB):
        if ib + 1 < IB:
            fq.extend(ph1_units(ib + 1))
        emit_attn_block(ib, fill)
        while fq:
            fq.pop(0)()
    emit_proj_dt(IB - 1, 1, 0)


def build():
    if "nc" in _CACHE:
        return _CACHE["nc"]
    nc = bacc.Bacc(
        "TRN2", target_bir_lowering=False, debug=False, num_devices=N_CORES
    )
    aps = {
        "xt": nc.dram_tensor("xt", [128, KT, T], CD, kind="ExternalInput").ap(),
        "wqkv": nc.dram_tensor("wqkv", [128, KT, 6, 128], CD, kind="ExternalInput").ap(),
        "bqk": nc.dram_tensor("bqk", [128, 4], F32, kind="ExternalInput").ap(),
        "bv": nc.dram_tensor("bv", [1, 256], F32, kind="ExternalInput").ap(),
        "wp": nc.dram_tensor("wp", [128, 2, C], CD, kind="ExternalInput").ap(),
        "out": nc.dram_tensor("out", [T + 512, C], CD, kind="ExternalOutput").ap(),
    }
    from contextlib import ExitStack

    with tile.TileContext(nc) as tc:
        with ExitStack() as ctx:
            _emit(nc, tc, ctx, aps)
    nc.compile()
    _CACHE["nc"] = nc
    return nc


def make_in_maps(x, W_attn, b_attn, W_proj, b_proj):
    x = np.asarray(x, dtype=np.float32)
    W_attn = np.asarray(W_attn, dtype=np.float32)
    b_attn = np.asarray(b_attn, dtype=np.float32)
    W_proj = np.asarray(W_proj, dtype=np.float32)

    in_maps = []
    xt_b = {}
    for b in range(B):
        xt = np.ascontiguousarray(x[b].T)  # [C, T]
        xt_b[b] = (
            xt.reshape(KT, 128, T).transpose(1, 0, 2).astype(CD_NP)
        )
    for core in range(N_CORES):
        b = core // 4
        g = core % 4
        fs = slice(256 * g, 256 * g + 256)  # feature cols for this head group
        wq = W_attn[:, fs]
        wk = W_attn[:, C + 256 * g: C + 256 * g + 256]
        wv = W_attn[:, 2 * C + 256 * g: 2 * C + 256 * g + 256]
        wqkv = np.concatenate([wq, wk, wv], axis=1)  # [1024, 768]
        bq = b_attn[fs]
        bk = b_attn[C + 256 * g: C + 256 * g + 256]
        bv = b_attn[2 * C + 256 * g: 2 * C + 256 * g + 256]
        in_maps.append({
            "xt": xt_b[b],
            "wqkv": np.ascontiguousarray(
                wqkv.reshape(KT, 128, 6, 128).transpose(1, 0, 2, 3)
            ).astype(CD_NP),
            "bqk": np.ascontiguousarray(
                np.concatenate([bq, bk]).reshape(4, 128).T
            ).astype(np.float32),
            "bv": bv[None, :].astype(np.float32),
            "wp": np.ascontiguousarray(
                W_proj[fs, :].reshape(2, 128, C).transpose(1, 0, 2)
            ).astype(CD_NP),
        })
    return in_maps


def _ensure_ntff_hook():
    """Recreate the missing antenv.axon_hooks NTFF-profile shim (see
    trn_agent_boot/trn_boot.py) so run_bass_kernel_spmd(trace=True) works."""
    import contextlib
    import ctypes
    import types

    try:
        from antenv.axon_hooks import get_axon_ntff_profile_hook  # noqa: F401

        return
    except ImportError:
        pass

    mod = types.ModuleType("antenv.axon_hooks")
    _holder = {"hook": None}
    mod.set_axon_ntff_profile_hook = lambda h: _holder.__setitem__("hook", h)
    mod.get_axon_ntff_profile_hook = lambda: _holder["hook"]
    sys.modules["antenv.axon_hooks"] = mod
    import antenv

    antenv.axon_hooks = mod

    so_path = "/opt/axon/libaxon_pjrt.so"
    if not os.path.exists(so_path):
        return
    lib = ctypes.CDLL(so_path)
    if not hasattr(lib, "axon_start_nrt_profile"):
        return
    lib.axon_start_nrt_profile.argtypes = [
        ctypes.POINTER(ctypes.c_int64),
        ctypes.c_size_t,
    ]
    lib.axon_start_nrt_profile.restype = ctypes.c_int64
    lib.axon_stop_nrt_profile.argtypes = [ctypes.c_char_p]
    lib.axon_stop_nrt_profile.restype = ctypes.c_int64

    @contextlib.contextmanager
    def _hook(output_dir, device_ids):
        import jax

        jax.devices()
        if device_ids:
            ids = (ctypes.c_int64 * len(device_ids))(*device_ids)
            rc = lib.axon_start_nrt_profile(ids, len(device_ids))
        else:
            rc = lib.axon_start_nrt_profile(None, 0)
        if rc != 0:
            raise RuntimeError(f"axon_start_nrt_profile rc={rc}")
        try:
            yield
        finally:
            n = lib.axon_stop_nrt_profile(str(output_dir).encode())
            if n <= 0:
                print(f"ntff profile: rc={n}, nothing written to {output_dir}")

    mod.set_axon_ntff_profile_hook(_hook)


def kernel(x, W_attn, b_attn, W_proj, b_proj):
    global LAST_RESULT
    nc = build()
    in_maps = make_in_maps(x, W_attn, b_attn, W_proj, b_proj)
    b_proj = np.asarray(b_proj, dtype=np.float32)
    trace = os.environ.get("KERNEL_TRACE", "0") == "1"
    if trace:
        _ensure_ntff_hook()
        import concourse.bass_utils as _bu

        _bu.upload_artifacts = lambda tmpdir: f"local://{tmpdir}"
    res = run_bass_kernel_spmd(
        nc, in_maps, core_ids=list(range(N_CORES)), trace=trace
    )
    LAST_RESULT = res
    outs = [res.results[i]["out"] for i in range(N_CORES)]
    y = np.empty((B, T, C), dtype=np.float32)
    for b in range(B):
        acc = outs[4 * b].astype(np.float32)
        for g in range(1, 4):
            acc = acc + outs[4 * b + g].astype(np.float32)
        y[b] = acc[:T] + b_proj
        y[b][T - 512:T] += acc[T:T + 512]
    return y


# revision 28
# speedup vs baseline: 1.0389x; 1.0389x over previous
"""Causal self-attention (B=2, T=2048, C=1024, nh=16) on 8 TRN2 NeuronCores.

Sharding: core c -> batch b = c//4, head group g = c%4 (4 heads each).
Each core computes QKV projections for its heads, causal attention, and a
partial output projection (W_proj rows for its heads). The four partials per
batch are summed on the host, which also adds b_proj.

Layouts (per core, hardcoded):
  xt    [128, 8, 2048]     x[b].T tiles:  xt[p, kt, t] = x[b, t, kt*128+p]
  wqkv  [128, 8, 6, 128]   W_attn q|k|v cols for this core's heads
  bqk   [128, 4] f32       b_attn q|k (per-partition bias)
  bv    [1, 256] f32       b_attn v (broadcast on device)
  wp    [128, 2, 1024]     W_proj rows for this core's heads
  out   [2048, 1024] bf16  partial (x[b] @ ... for this head group)

In-kernel dataflow (all matmuls bf16 with fp32 PSUM accumulation), emitted
tb-block-outer so attention for i-block ib starts right after phase-1 block
tb=ib finishes:
  qT,kT = (W.T @ x.T)      [feat, t] layout  (lhsT=W tile, rhs=xT)
  v     = (x @ Wv)         [t, feat] layout  (lhsT=xT t-tile, rhs=Wv cols)
          -> written straight into vext (65-col per head, ones column)
  S^T   = k @ q.T          [j, i] layout     (lhsT=kT tile, rhs=qT)
  P^T   = exp(S^T/8), masked on diagonal tiles (mult by 0/1 triangle)
  y^T,l = [v|1].T @ P^T    [d, i] layout, row 64 = l = sum_j P
  yT    = y^T * (1/l broadcast)
  out   = yT.T @ Wp        (bias added on host)
"""

import os
import sys

sys.path.insert(0, "/opt/trn_rl_repo")
os.environ.setdefault("MYCRO_LOCAL_CACHE", "1")

import ml_dtypes
import numpy as np

import concourse.bass as bass
import concourse.mybir as mybir
import concourse.tile as tile
from concourse import bacc
from concourse.bass_utils import run_bass_kernel_spmd

B, T, C, NH, HS = 2, 2048, 1024, 16, 64
HPC = 4  # heads per core
N_CORES = 8
KT = C // 128  # 8 contraction tiles over C
TT = T // 128  # 16 tiles over T
IB = T // 512  # 4 i-blocks over T
F32 = mybir.dt.float32

CD = mybir.dt.bfloat16
CD_NP = ml_dtypes.bfloat16

LAST_RESULT = None
_CACHE = {}


def _emit(nc, tc, ctx, aps):
    xt, wqkv, bqk, bv, wp, out = (
        aps["xt"], aps["wqkv"], aps["bqk"], aps["bv"], aps["wp"], aps["out"],
    )
    Exp = mybir.ActivationFunctionType.Exp

    consts = ctx.enter_context(tc.tile_pool(name="consts", bufs=1))

    # --- PE warm-up: full-K dummy matmuls keep HAM busy from t~0 so real
    # matmuls run at 2.4 GHz as soon as their input DMAs land. (K<128
    # row-masked matmuls do NOT count as PE-busy for the HAM clock gate.)
    dummy = consts.tile([128, 512], CD, tag="dummy")
    nc.vector.memset(dummy[:], 1.0)

    # --- persistent SBUF tensors. DMAs are split into ~256-512KB pieces and
    # issued round-robin so all 8 DMA rings transfer in parallel (a single
    # ring moves only ~50-70 GB/s; one 1.5MB DMA alone costs 20-30us).
    # tb=0 pieces and weights go first so phase-1 can start at ~8us.
    wqkv_s = consts.tile([128, KT, 6, 128], CD, tag="wqkv")
    xt_s = consts.tile([128, KT, T], CD, tag="xt")
    bqk_s = consts.tile([128, 4], F32, tag="bqk")
    bv_row = consts.tile([1, 256], F32, tag="bv_row")
    wp_s = consts.tile([128, 2, C], CD, tag="wp")
    for kq in range(4):
        ks = slice(2 * kq, 2 * kq + 2)
        nc.sync.dma_start(xt_s[:, ks, 0:512], xt[:, ks, 0:512])
        nc.sync.dma_start(wqkv_s[:, ks], wqkv[:, ks])
    # small tensors ride right behind the first wave; tb>=1 x chunks after
    nc.sync.dma_start(bqk_s[:], bqk)
    nc.sync.dma_start(bv_row[:], bv)
    nc.sync.dma_start(wp_s[:], wp)
    for tb in range(1, IB):
        tsl = slice(tb * 512, (tb + 1) * 512)
        for hf in range(2):
            ks = slice(4 * hf, 4 * hf + 4)
            nc.sync.dma_start(xt_s[:, ks, tsl], xt[:, ks, tsl])

    # fixed 128x128 causal triangle (keep where j <= c) for diagonal strips
    tri = consts.tile([128, 128], CD, tag="tri")
    nc.vector.memset(tri[:], 1.0)
    nc.gpsimd.affine_select(
        out=tri[:],
        in_=tri[:],
        compare_op=mybir.AluOpType.is_ge,
        fill=0.0,
        base=0,
        channel_multiplier=-1,
        pattern=[[1, 128]],
    )

    qk_t = [consts.tile([128, T], CD, tag=f"q{jt}", name=f"q{jt}")
            for jt in range(2)]
    # kT per head, zero-padded to full 128 partitions: head h occupies rows
    # (h%2)*64..+64, the other 64 rows stay zero. Full-K S-matmuls keep the
    # PE HAM clock-gate warm (K=64 row-masked MMs don't count as PE-busy).
    kz_t = [consts.tile([128, T], CD, tag=f"kz{h}", name=f"kz{h}")
            for h in range(HPC)]
    for h in range(HPC):
        row = ((h + 1) % 2) * 64  # rows NOT written by the k projection
        eng = nc.vector if h < 2 else nc.gpsimd
        eng.memset(kz_t[h][row:row + 64, :], 0.0)
    vext_s = consts.tile([128, TT, HPC * (HS + 1)], CD, tag="vext")
    vext4 = vext_s[:].rearrange("p t (h c) -> p t h c", c=HS + 1)
    nc.gpsimd.memset(vext4[:, :, :, HS], 1.0)  # ones columns
    yt_s = consts.tile([128, 2, T], CD, tag="yt")
    # broadcast LAST on the gpsimd queue: it waits on the bv DMA, and the
    # in-order queue would otherwise block tri/memsets behind it
    bv_bc = consts.tile([128, 256], F32, tag="bv_bc")
    nc.gpsimd.partition_broadcast(bv_bc[:], bv_row[:], channels=128)

    # pools: mm512 is shared by phase-1 QKV groups and the proj matmuls
    mm512 = ctx.enter_context(tc.tile_pool(name="mm512", bufs=2, space="PSUM"))
    attn_sp = ctx.enter_context(tc.tile_pool(name="attn_s", bufs=2, space="PSUM"))
    attn_yp = ctx.enter_context(tc.tile_pool(name="attn_y", bufs=2, space="PSUM"))
    pt_pool = ctx.enter_context(tc.tile_pool(name="pt", bufs=6))
    misc = ctx.enter_context(tc.tile_pool(name="misc", bufs=4))
    stage = ctx.enter_context(tc.tile_pool(name="stage", bufs=3))

    # warm-up matmuls (K=128, N=512): ~3.5us of PE busy before real work
    warm_ps = mm512.tile([128, 512], F32, tag="mm")
    for _ in range(8):
        nc.tensor.matmul(
            out=warm_ps[:], lhsT=dummy[:, 0:128], rhs=dummy[:],
            start=True, stop=True,
        )
    # tiny exp up front pulls the ~2.7us ACT_TABLE_LOAD into the DMA wait
    act_warm = misc.tile([1, 16], CD, tag="actw")
    nc.scalar.activation(out=act_warm[:], in_=dummy[0:1, 0:16], func=Exp)

    def ph1_units(tb):
        """Phase-1 block for token block tb, split into half-group units
        (4 matmuls each) that can braid into attention's exp-paced windows.
        k groups first so the k->bias-add->S chain overlaps the q groups."""
        tsl = slice(tb * 512, (tb + 1) * 512)
        units = []

        def qk_unit(jt, half, cell):
            def emit():
                if half == 0:
                    cell["ps"] = mm512.tile(
                        [128, 512], F32, tag="mm", name=f"qk{tb}_{jt}"
                    )
                ps = cell["ps"]
                for kt in range(4 * half, 4 * half + 4):
                    nc.tensor.matmul(
                        out=ps[:],
                        lhsT=wqkv_s[:, kt, jt, :],
                        rhs=xt_s[:, kt, tsl],
                        start=(kt == 0),
                        stop=(kt == KT - 1),
                    )
                if half == 0:
                    return
                if jt < 2:  # q
                    nc.vector.tensor_scalar_add(
                        qk_t[jt][:, tsl], ps[:], bqk_s[:, jt:jt + 1]
                    )
                else:  # k -> zero-padded per-head kz
                    nc.vector.tensor_scalar_add(
                        kz_t[2 * (jt - 2)][0:64, tsl],
                        ps[0:64, :],
                        bqk_s[0:64, jt:jt + 1],
                    )
                    nc.vector.tensor_scalar_add(
                        kz_t[2 * (jt - 2) + 1][64:128, tsl],
                        ps[64:128, :],
                        bqk_s[64:128, jt:jt + 1],
                    )
            return emit

        def v_unit(tt, half, cell):
            def emit():
                if half == 0:
                    cell["ps"] = mm512.tile(
                        [128, 512], F32, tag="mm", name=f"v{tt}"
                    )
                ps = cell["ps"]
                for kt in range(4 * half, 4 * half + 4):
                    nc.tensor.matmul(
                        out=ps[:, 0:256],
                        lhsT=xt_s[:, kt, tt * 128:(tt + 1) * 128],
                        rhs=wqkv_s[:, kt, 4:6, :].rearrange("p a b -> p (a b)"),
                        start=(kt == 0),
                        stop=(kt == KT - 1),
                    )
                if half == 1:
                    nc.vector.tensor_tensor(
                        out=vext4[:, tt, :, 0:HS],
                        in0=ps[:, 0:256].rearrange("p (h c) -> p h c", c=HS),
                        in1=bv_bc[:].rearrange("p (h c) -> p h c", c=HS),
                        op=mybir.AluOpType.add,
                    )
            return emit

        for jt in (2, 3, 0, 1):  # 2,3 = k; 0,1 = q
            cell = {}
            units += [qk_unit(jt, 0, cell), qk_unit(jt, 1, cell)]
        for tt in range(tb * 4, tb * 4 + 4):
            cell = {}
            units += [v_unit(tt, 0, cell), v_unit(tt, 1, cell)]
        return units

    def proj_units(ib):
        units = []

        def unit(tloc):
            def emit():
                emit_proj_tloc(ib, tloc)
            return emit

        for tloc in range(4):
            units.append(unit(tloc))
        return units

    def emit_proj_tloc(ib, tloc):
        ttp = ib * 4 + tloc
        st = stage.tile([128, C], CD, tag="st", name=f"st{ttp}")
        for eb in range(2):
            psp = mm512.tile([128, 512], F32, tag="mm", name=f"pj{ttp}_{eb}")
            for dt in range(2):
                nc.tensor.matmul(
                    out=psp[:],
                    lhsT=yt_s[:, dt, ttp * 128:(ttp + 1) * 128],
                    rhs=wp_s[:, dt, eb * 512:(eb + 1) * 512],
                    start=(dt == 0),
                    stop=(dt == 1),
                )
            nc.vector.tensor_copy(st[:, eb * 512:(eb + 1) * 512], psp[:])
        nc.sync.dma_start(out[ttp * 128:(ttp + 1) * 128, :], st[:])

    def emit_proj_block(ib):
        for tloc in range(4):
            emit_proj_tloc(ib, tloc)

    def emit_pv_quad(layout, pt, psy, h, last_quad):
        for idx, (j, c, w, off) in enumerate(layout):
            nc.tensor.matmul(
                out=psy[:, off:512],
                lhsT=vext4[:, j, h, :],
                rhs=pt[:, c:c + w],
                start=(j == 0),
                stop=(last_quad and idx == len(layout) - 1),
            )

    def emit_attn_block(ib, fill):
        isl = slice(ib * 512, (ib + 1) * 512)
        njt = 4 * ib + 4
        for h in range(HPC):
            jt_q = h // 2
            row = (h % 2) * 64
            psy = attn_yp.tile([HS + 1, 512], F32, tag="y")
            prev = None
            # one exp per pair of j-tiles (2-bank PSUM read); PV of the
            # previous pair runs on the PE while this pair's exp is on the
            # scalar engine
            for g in range(njt // 2):
                psS = attn_sp.tile([128, 1024], F32, tag="s")
                pt = pt_pool.tile([128, 1024], CD, tag="pt")
                js = [2 * g, 2 * g + 1]
                layout = []
                cs = 0
                for j in js:
                    off = max(0, 128 * j - 512 * ib)
                    w = 512 - off
                    if (cs % 512) + w > 512:  # keep each MM within one bank
                        cs = (cs + 511) // 512 * 512
                    nc.tensor.matmul(
                        out=psS[:, cs:cs + w],
                        lhsT=kz_t[h][:, j * 128:(j + 1) * 128],
                        rhs=qk_t[jt_q][:, ib * 512 + off:(ib + 1) * 512],
                        start=True,
                        stop=True,
                    )
                    layout.append((j, cs, w, off))
                    cs += w
                nc.scalar.activation(
                    out=pt[:, 0:cs], in_=psS[:, 0:cs], func=Exp, scale=0.125,
                )
                for (j, c, w, off) in layout:
                    if 128 * j >= 512 * ib:  # diagonal tile -> mask boundary
                        nc.vector.tensor_mul(
                            pt[:, c:c + 128], pt[:, c:c + 128], tri[:],
                        )
                if prev is not None:
                    emit_pv_quad(prev[1], prev[0], psy, h, False)
                fill()
                prev = (pt, layout)
            emit_pv_quad(prev[1], prev[0], psy, h, True)
            lrow = misc.tile([1, 512], F32, tag="lrow")
            nc.vector.tensor_copy(lrow[:], psy[HS:HS + 1, :])
            linv = misc.tile([1, 512], F32, tag="linv")
            nc.vector.reciprocal_approx_fast(linv[:], lrow[:])
            lbc = misc.tile([64, 512], F32, tag="lbc")
            nc.gpsimd.partition_broadcast(lbc[:], linv[:], channels=64)
            nc.vector.tensor_mul(
                yt_s[row:row + 64, jt_q, isl], psy[0:HS, :], lbc[:]
            )
            if h == 0 and ib > 0:
                # previous i-block's projection: its yt is long-ready; braid
                # its chunks into the exp-paced PE windows of heads 1-3
                # rather than clumping them here (the clump starves the
                # scalar engine of S tiles for ~4us)
                fq.extend(proj_units(ib - 1))

    # interleave: phase-1 block 0 up front, then attention i-block ib with
    # phase-1 block ib+1 braided into its exp-paced PE windows
    fq = []

    def fill():
        if fq:
            fq.pop(0)()

    for u in ph1_units(0):
        u()
    for ib in range(IB):
        if ib + 1 < IB:
            fq.extend(ph1_units(ib + 1))
        emit_attn_block(ib, fill)
        while fq:
            fq.pop(0)()
    emit_proj_block(IB - 1)


def build():
    if "nc" in _CACHE:
        return _CACHE["nc"]
    nc = bacc.Bacc(
        "TRN2", target_bir_lowering=False, debug=False, num_devices=N_CORES
    )
    aps = {
        "xt": nc.dram_tensor("xt", [128, KT, T], CD, kind="ExternalInput").ap(),
        "wqkv": nc.dram_tensor("wqkv", [128, KT, 6, 128], CD, kind="ExternalInput").ap(),
        "bqk": nc.dram_tensor("bqk", [128, 4], F32, kind="ExternalInput").ap(),
        "bv": nc.dram_tensor("bv", [1, 256], F32, kind="ExternalInput").ap(),
        "wp": nc.dram_tensor("wp", [128, 2, C], CD, kind="ExternalInput").ap(),
        "out": nc.dram_tensor("out", [T, C], CD, kind="ExternalOutput").ap(),
    }
    from contextlib import ExitStack

    with tile.TileContext(nc) as tc:
        with ExitStack() as ctx:
            _emit(nc, tc, ctx, aps)
    nc.compile()
    _CACHE["nc"] = nc
    return nc


def make_in_maps(x, W_attn, b_attn, W_proj, b_proj):
    x = np.asarray(x, dtype=np.float32)
    W_attn = np.asarray(W_attn, dtype=np.float32)
    b_attn = np.asarray(b_attn, dtype=np.float32)
    W_proj = np.asarray(W_proj, dtype=np.float32)

    in_maps = []
    xt_b = {}
    for b in range(B):
        xt = np.ascontiguousarray(x[b].T)  # [C, T]
        xt_b[b] = (
            xt.reshape(KT, 128, T).transpose(1, 0, 2).astype(CD_NP)
        )
    for core in range(N_CORES):
        b = core // 4
        g = core % 4
        fs = slice(256 * g, 256 * g + 256)  # feature cols for this head group
        wq = W_attn[:, fs]
        wk = W_attn[:, C + 256 * g: C + 256 * g + 256]
        wv = W_attn[:, 2 * C + 256 * g: 2 * C + 256 * g + 256]
        wqkv = np.concatenate([wq, wk, wv], axis=1)  # [1024, 768]
        bq = b_attn[fs]
        bk = b_attn[C + 256 * g: C + 256 * g + 256]
        bv = b_attn[2 * C + 256 * g: 2 * C + 256 * g + 256]
        in_maps.append({
            "xt": xt_b[b],
            "wqkv": np.ascontiguousarray(
                wqkv.reshape(KT, 128, 6, 128).transpose(1, 0, 2, 3)
            ).astype(CD_NP),
            "bqk": np.ascontiguousarray(
                np.concatenate([bq, bk]).reshape(4, 128).T
            ).astype(np.float32),
            "bv": bv[None, :].astype(np.float32),
            "wp": np.ascontiguousarray(
                W_proj[fs, :].reshape(2, 128, C).transpose(1, 0, 2)
            ).astype(CD_NP),
        })
    return in_maps


def _ensure_ntff_hook():
    """Recreate the missing antenv.axon_hooks NTFF-profile shim (see
    trn_agent_boot/trn_boot.py) so run_bass_kernel_spmd(trace=True) works."""
    import contextlib
    import ctypes
    import types

    try:
        from antenv.axon_hooks import get_axon_ntff_profile_hook  # noqa: F401

        return
    except ImportError:
        pass

    mod = types.ModuleType("antenv.axon_hooks")
    _holder = {"hook": None}
    mod.set_axon_ntff_profile_hook = lambda h: _holder.__setitem__("hook", h)
    mod.get_axon_ntff_profile_hook = lambda: _holder["hook"]
    sys.modules["antenv.axon_hooks"] = mod
    import antenv

    antenv.axon_hooks = mod

    so_path = "/opt/axon/libaxon_pjrt.so"
    if not os.path.exists(so_path):
        return
    lib = ctypes.CDLL(so_path)
    if not hasattr(lib, "axon_start_nrt_profile"):
        return
    lib.axon_start_nrt_profile.argtypes = [
        ctypes.POINTER(ctypes.c_int64),
        ctypes.c_size_t,
    ]
    lib.axon_start_nrt_profile.restype = ctypes.c_int64
    lib.axon_stop_nrt_profile.argtypes = [ctypes.c_char_p]
    lib.axon_stop_nrt_profile.restype = ctypes.c_int64

    @contextlib.contextmanager
    def _hook(output_dir, device_ids):
        import jax

        jax.devices()
        if device_ids:
            ids = (ctypes.c_int64 * len(device_ids))(*device_ids)
            rc = lib.axon_start_nrt_profile(ids, len(device_ids))
        else:
            rc = lib.axon_start_nrt_profile(None, 0)
        if rc != 0:
            raise RuntimeError(f"axon_start_nrt_profile rc={rc}")
        try:
            yield
        finally:
            n = lib.axon_stop_nrt_profile(str(output_dir).encode())
            if n <= 0:
                print(f"ntff profile: rc={n}, nothing written to {output_dir}")

    mod.set_axon_ntff_profile_hook(_hook)


def kernel(x, W_attn, b_attn, W_proj, b_proj):
    global LAST_RESULT
    nc = build()
    in_maps = make_in_maps(x, W_attn, b_attn, W_proj, b_proj)
    b_proj = np.asarray(b_proj, dtype=np.float32)
    trace = os.environ.get("KERNEL_TRACE", "0") == "1"
    if trace:
        _ensure_ntff_hook()
        import concourse.bass_utils as _bu

        _bu.upload_artifacts = lambda tmpdir: f"local://{tmpdir}"
    res = run_bass_kernel_spmd(
        nc, in_maps, core_ids=list(range(N_CORES)), trace=trace
    )
    LAST_RESULT = res
    outs = [res.results[i]["out"] for i in range(N_CORES)]
    y = np.empty((B, T, C), dtype=np.float32)
    for b in range(B):
        acc = outs[4 * b].astype(np.float32)
        for g in range(1, 4):
            acc = acc + outs[4 * b + g].astype(np.float32)
        y[b] = acc + b_proj
    return y
